# revision 51
# baseline (speedup 1.0000x reference)
"""Cross-Scale Non-Local Attention kernel for 8x Trainium2 NeuronCores.

Data-parallel over batch: each of the 8 cores processes one sample
(B=8, H=W=64, C=64). Per-core Bass/Tile program (v2 schedule):

  1. x loaded in 4 chunks; each chunk PE-transposed to channel-major
     xT [65, 4096] whose last partition row is ones, so every 1x1 conv
     bias folds into its matmul (K=65).
  2. g = prelu(xT.T @ g_w) bounced per-x-chunk to DRAM as bf16 g_poly
     (polyphase layout, zero border ring); the 18 shifted dynamic-filter
     views kg[q,qw,kb] [n=128, 4, 256] are gathered back by strided DMA.
  3. theta 1x1 convs (4x-replicated weights) produce per-chunk t_act
     tiles; the 9 shift-baked pack copies into thetaPackA/B/C are
     DEFERRED into the deconv pipeline (prefetch distance 2) where the
     DVE/GpSimd/Scalar engines are otherwise idle, instead of forming a
     60us vector-bound wall before the deconv as in v1.
  4. phi is accumulated per-x-chunk during the transposes (bilinear
     4-tap partials), its 0.25 scale folded into phi_w; the patch-norm
     3x3 box sum is separable (4 adds), the reciprocal uses the fast
     approx DVE op. phi tap packs get the softmax scale folded in.
  5. Main pipeline, lag-2 (deconv(pc) tile-waits on every issued B;
     each B's off-PE chain is hidden by the h1+scores matmuls in front
     of its consumer): per iteration
     scores/Exp(ch) -> deconv(ch-2) first half -> packs(ch+2) ->
     B(ch-1) -> deconv(ch-2) second half. The softmax denominator
     broadcast is a K=1 matmul into psum (gpsimd partition_broadcast
     costs ~7us in program switches), the reciprocal is the fast
     approx DVE op, and keepalive matmuls keep the HAM clock gate open
     until the deconv stream sustains it.
  6. Deconv as polyphase conv-transpose: psum[pix 128, rc 512]
     accumulates 18 shifted bf16 matmuls; drain = one scalar copy
     (fp32->bf16) + two output DMAs per tile (y is written bf16 and
     widened on host, halving drain bytes and DMA descriptor size).
"""

import numpy as np

_CACHE = {}

# Problem constants (hardcoded per harness contract)
B = 8
H = W = 64
C = 64
CI = 32
HS = WS = 16
N = 256          # HS*WS low-res positions
PH = 66          # padded attn spatial extent (64 + 1 halo each side)

# tap order for the packed scores operands: packs A,B hold 4 taps each
# on partition groups 0-3; pack C holds the 9th tap on partitions 0-31
TAPS = [(0, 0), (0, 1), (0, 2), (1, 0), (1, 1), (1, 2), (2, 0), (2, 1),
        (2, 2)]


def _build_nc():
    import concourse.bass as bass
    import concourse.tile as tile
    from concourse import bacc, mybir
    from concourse.masks import make_identity
    from contextlib import ExitStack

    F32 = mybir.dt.float32
    F32R = mybir.dt.float32r
    BF16 = mybir.dt.bfloat16
    Alu = mybir.AluOpType
    Act = mybir.ActivationFunctionType

    def r_(ap):
        return ap.bitcast(F32R)

    nc = bacc.Bacc("TRN2", debug=False)

    x_h = nc.dram_tensor("x", [H, W, C], F32, kind="ExternalInput")
    thw_h = nc.dram_tensor("theta_w", [C, CI], F32, kind="ExternalInput")
    thb_h = nc.dram_tensor("theta_b", [CI], F32, kind="ExternalInput")
    tha_h = nc.dram_tensor("theta_alpha", [CI], F32, kind="ExternalInput")
    phw_h = nc.dram_tensor("phi_w", [C, CI], F32, kind="ExternalInput")
    phb_h = nc.dram_tensor("phi_b", [CI], F32, kind="ExternalInput")
    pha_h = nc.dram_tensor("phi_alpha", [CI], F32, kind="ExternalInput")
    gw_h = nc.dram_tensor("g_w", [C, C], F32, kind="ExternalInput")
    gb_h = nc.dram_tensor("g_b", [C], F32, kind="ExternalInput")
    ga_h = nc.dram_tensor("g_alpha", [C], F32, kind="ExternalInput")
    y_h = nc.dram_tensor("y", [4 * H, 4 * W, C], BF16, kind="ExternalOutput")

    with tile.TileContext(nc) as tc, ExitStack() as top:
        ec = top.enter_context

        consts = ec(tc.tile_pool(name="consts", bufs=1))
        xp_pool = ec(tc.tile_pool(name="xp_pool", bufs=1))
        persist = ec(tc.tile_pool(name="persist", bufs=1))
        phip = ec(tc.tile_pool(name="phip", bufs=1))
        dramp = ec(tc.tile_pool(name="dramp", bufs=1, space="DRAM"))
        staging = ec(tc.tile_pool(name="staging", bufs=3))
        ps_sc = ec(tc.tile_pool(name="ps_sc", bufs=2, space="PSUM"))
        ps_Sp = ec(tc.tile_pool(name="ps_Sp", bufs=1, space="PSUM"))
        ps_ka = ec(tc.tile_pool(name="ps_ka", bufs=1, space="PSUM"))

        # ---- x first on the DMA queues ----
        xP = xp_pool.tile([128, 32, C], F32)
        x_r = x_h.ap().rearrange("h w c -> (h w) c").rearrange(
            "(t p) c -> p t c", p=128)
        for xc in range(4):
            nc.sync.dma_start(
                out=xP[:, xc * 8:(xc + 1) * 8, :],
                in_=x_r[:, xc * 8:(xc + 1) * 8, :])

        # ---- constants / weights in SBUF ----
        ident = consts.tile([128, 128], F32)
        make_identity(nc, ident)
        # HAM warmup: bf16 matmuls keep the PE busy through the initial
        # x-load DMA so the clock gate is released when real work starts.
        wu = consts.tile([128, 512], BF16)
        nc.vector.memset(wu, 0.0)
        ps_wu = ps_ka.tile([128, 512], F32, tag="ka", name="ps_wu")
        for i in range(16):
            nc.tensor.matmul(ps_wu, wu[:, :128], wu, start=True, stop=True)

        def keepalive(n):
            for _ in range(n):
                nc.tensor.matmul(ps_wu, wu[:, :128], wu,
                                 start=True, stop=True)

        # weights with the bias as a 65th contraction row; theta/phi
        # replicated 4x along the output dim so the PE emits the
        # (tap-group, ci) packed partitions directly
        # weight loads go out on the scalar engine's DMA queue so the
        # sync queue reaches the g bounce + kg gathers sooner
        thw4 = consts.tile([C + 1, 128], F32)
        phw4 = consts.tile([C + 1, 128], F32)
        gw_sb = consts.tile([C + 1, C], F32)
        thw_sb = consts.tile([C, CI], F32)
        nc.scalar.dma_start(out=r_(thw_sb), in_=r_(thw_h.ap()))
        phw_sb = consts.tile([C, CI], F32)
        nc.scalar.dma_start(out=r_(phw_sb), in_=r_(phw_h.ap()))
        nc.scalar.dma_start(out=r_(gw_sb[:C]), in_=r_(gw_h.ap()))
        nc.scalar.dma_start(out=r_(gw_sb[C:C + 1, :]),
                            in_=r_(gb_h.ap().unsqueeze(0)))
        tha4 = consts.tile([128, 1], F32)
        pha4 = consts.tile([128, 1], F32)
        for g in range(4):
            sl = slice(32 * g, 32 * g + 32)
            nc.scalar.dma_start(out=tha4[sl], in_=tha_h.ap().unsqueeze(1))
            nc.scalar.dma_start(out=pha4[sl], in_=pha_h.ap().unsqueeze(1))
            nc.scalar.dma_start(out=r_(thw4[C:C + 1, sl]),
                                in_=r_(thb_h.ap().unsqueeze(0)))
            nc.scalar.dma_start(out=r_(phw4[C:C + 1, sl]),
                                in_=r_(phb_h.ap().unsqueeze(0)))
        for g in range(4):
            nc.vector.tensor_copy(out=r_(thw4[:C, 32 * g:32 * g + 32]),
                                  in_=thw_sb)
            nc.vector.tensor_copy(out=r_(phw4[:C, 32 * g:32 * g + 32]),
                                  in_=phw_sb)
        # fold the bilinear 0.25 into phi_w (not the bias row)
        nc.vector.tensor_scalar_mul(r_(phw4[:C]), phw4[:C], 0.25)
        ga_row = consts.tile([1, C], F32)
        nc.scalar.dma_start(out=ga_row, in_=ga_h.ap().unsqueeze(0))
        ga_row8 = consts.tile([1, 8 * C], F32)
        for t in range(8):
            nc.vector.tensor_copy(out=ga_row8[:, t * C:(t + 1) * C],
                                  in_=ga_row)
        ga8 = consts.tile([128, 8 * C], F32)
        nc.gpsimd.partition_broadcast(ga8, ga_row8)
        sixes128 = consts.tile([128, 1], BF16)
        nc.vector.memset(sixes128, 6.0)
        ones32 = consts.tile([CI, 1], F32)
        nc.vector.memset(ones32, 1.0)
        # [1,128] ones: K=1 matmul broadcasts a row across partitions
        # (the gpsimd partition_broadcast costs ~7us in program switches)
        ones_row = consts.tile([1, 128], F32)
        nc.vector.memset(ones_row, 1.0)
        ln10 = consts.tile([1, 1], F32)
        nc.vector.memset(ln10, 2.302585092994046)
        ztb16 = consts.tile([128, 1024], BF16)
        nc.vector.memset(ztb16, 0.0)

        # polyphase g layouts in DRAM; the zero border ring only needs
        # ztb16, so its fills go out with the startup DMAs
        g_poly = dramp.tile([18, 18, 4, 256], BF16)
        g_lin = dramp.tile([H, W, C], BF16)
        gp_r0 = g_poly[0].rearrange("b r x -> (b r x)").rearrange(
            "(p f) -> p f", f=512)
        nc.scalar.dma_start(out=gp_r0, in_=ztb16[:36, :512])
        gp_r1 = g_poly[17].rearrange("b r x -> (b r x)").rearrange(
            "(p f) -> p f", f=512)
        nc.scalar.dma_start(out=gp_r1, in_=ztb16[:36, :512])
        gp_c0 = g_poly[1:17, 0].rearrange("a r x -> a (r x)")
        nc.scalar.dma_start(out=gp_c0, in_=ztb16[:16])
        gp_c1 = g_poly[1:17, 17].rearrange("a r x -> a (r x)")
        nc.scalar.dma_start(out=gp_c1, in_=ztb16[:16])

        # ---- persistent activation buffers ----
        attn_q = []
        for qw in range(3):
            t_ = persist.tile([128, 2, PH, 64], BF16, tag=f"attnq{qw}",
                              name=f"attnq{qw}")
            attn_q.append(t_)
        attn_q1n = persist.tile([128, 2, PH, 64], BF16, tag="attnq1n",
                                name="attnq1n")
        # raw E for all chunks (read-only after its Exp writes, so no
        # WAR chains)
        e_t = [persist.tile([128, 2, 8, 64], BF16, tag=f"e{ch}",
                            name=f"e{ch}") for ch in range(8)]
        # per-chunk theta activations, consumed by the deferred packs
        t_act = [persist.tile([128, 8, W], BF16, tag=f"ta{ch}",
                              name=f"ta{ch}") for ch in range(8)]
        thpA = persist.tile([128, 64, 64], BF16)
        thpB = persist.tile([128, 64, 64], BF16)
        thpC = persist.tile([CI, 64, 64], BF16)
        thp = [thpA, thpB, thpC]
        fpA = persist.tile([128, N], BF16)
        fpB = persist.tile([128, N], BF16)
        fpC = persist.tile([CI, N], BF16)

        phi4_pad = phip.tile([128, 18, 18], F32)
        nc.vector.memset(phi4_pad, 0.0)
        n2p = phip.tile([1, 324], F32)
        n2row = phip.tile([1, 16, 18], F32)
        nrm = phip.tile([1, N], F32)

        # only the never-written border strips of the packs / attn
        # planes need zeroing; issued up front (engines are idle during
        # the x load) as small zero-copies spread over three engines
        def zero_strips():
            strips = []
            for P, tile_, (g_lo, g_hi) in ((0, thpA, (0, 4)),
                                           (1, thpB, (4, 8)),
                                           (2, thpC, (8, 9))):
                for t in range(g_lo, g_hi):
                    kh, kw = TAPS[t]
                    g = t % 4
                    sl = slice(32 * g, 32 * g + 32)
                    if kh == 0:
                        strips.append((tile_[sl, 0, :], 64))
                    if kh == 2:
                        strips.append((tile_[sl, 63, :], 64))
                    if kw == 0:
                        strips.append((tile_[sl, :, 0], 64))
                    if kw == 2:
                        strips.append((tile_[sl, :, 63], 64))
            for tile2 in (attn_q[0], attn_q1n, attn_q[2]):
                for kb in range(2):
                    strips.append((tile2[:, kb, 0, :], 64))
                    strips.append((tile2[:, kb, PH - 1, :], 64))
            for kb in range(2):
                strips.append((attn_q[0][:, kb, :, 63], PH))
                strips.append((attn_q[2][:, kb, :, 0], PH))
            for i, (dst, width) in enumerate(strips):
                src = wu[:dst.shape[0], :width]
                e = i % 3
                if e == 0:
                    nc.vector.tensor_copy(out=dst, in_=src)
                elif e == 1:
                    nc.gpsimd.tensor_copy(out=dst, in_=src)
                else:
                    nc.scalar.copy(out=dst, in_=src)

        zero_strips()

        # bake each tap's spatial shift into its pack slice; scalar is
        # ~2x faster than DVE at these and gpsimd is slowest, so the
        # split leans on scalar with DVE relief
        PACK_ENG = ['s', 'v', 's', 's', 'v', 's', 's', 'v', 's']

        def packs(ch):
            h0 = ch * 8
            for t, (kh, kw) in enumerate(TAPS):
                P, g = t // 4, t % 4
                R0 = max(0, h0 + 1 - kh)
                R1 = min(64, h0 + 9 - kh)
                C0 = max(0, 1 - kw)
                C1 = min(64, 65 - kw)
                rs = R0 + kh - h0 - 1
                cs = C0 + kw - 1
                sl = slice(32 * g, 32 * g + 32)
                dst = thp[P][sl, R0:R1, C0:C1]
                src = t_act[ch][sl, rs:rs + R1 - R0, cs:cs + C1 - C0]
                eng = PACK_ENG[t]
                if eng == 'v':
                    nc.vector.tensor_copy(out=dst, in_=src)
                elif eng == 'g':
                    nc.gpsimd.tensor_copy(out=dst, in_=src)
                else:
                    nc.scalar.copy(out=dst, in_=src)

        with ExitStack() as st1:
            e1 = st1.enter_context
            xt_pool = e1(tc.tile_pool(name="xt_pool", bufs=1))
            gsb_pool = e1(tc.tile_pool(name="gsb_pool", bufs=1))
            ttmp = e1(tc.tile_pool(name="ttmp", bufs=2))
            gtmp = e1(tc.tile_pool(name="gtmp", bufs=2))
            ps_x = e1(tc.tile_pool(name="ps_x", bufs=1, space="PSUM"))
            ps_t = e1(tc.tile_pool(name="ps_t", bufs=1, space="PSUM"))
            ps_g = e1(tc.tile_pool(name="ps_g", bufs=2, space="PSUM"))

            # xT row 64 is all-ones: the bias row of the K=65 matmuls
            xT = xt_pool.tile([C + 1, H, W], F32)
            xTf = xT.rearrange("c h w -> c (h w)")
            nc.vector.memset(xT[C:C + 1], 1.0)
            phi_inT = xt_pool.tile([C + 1, HS, WS], F32)
            nc.vector.memset(phi_inT[C:C + 1], 1.0)
            g_sb = gsb_pool.tile([128, 32, C], BF16)

            def theta_chunk(ch):
                # the whole prelu is one scalar-engine parametric relu
                # (alpha is per-partition), freeing the DVE for g
                ps_tt = ps_t.tile([128, 512], F32, tag="t",
                                  name=f"ps_t{ch}")
                nc.tensor.matmul(
                    ps_tt, r_(thw4), r_(xTf[:, ch * 512:(ch + 1) * 512]),
                    start=True, stop=True)
                nc.scalar.activation(
                    out=t_act[ch].rearrange("p a b -> p (a b)"),
                    in_=ps_tt, func=Act.Prelu, alpha=tha4)

            # interleaved: transpose(t) -> copy -> g-mm(t-1) per pixel
            # chunk, so the g matmuls fill the PE gap left by the
            # psum->SBUF copy latency; keepalives hold the HAM clock up
            xv = xT.rearrange("c (hq hs) (wq ws) -> c hq hs wq ws",
                              hs=4, ws=4)
            gl5 = g_lin.rearrange(
                "(hq hr) (wq wr) c -> hq hr wq (wr c)", hr=4, wr=4)
            glint = g_lin.rearrange("(t a) w c -> a w t c", a=2)
            ps_g8s = {}

            def g_mm(t):
                xc = t // 8
                if xc not in ps_g8s:
                    ps_g8s[xc] = ps_g.tile([128, 8, C], F32, tag="g",
                                           name=f"ps_g8{xc}")
                nc.tensor.matmul(
                    ps_g8s[xc][:, t % 8, :],
                    r_(xTf[:, t * 128:(t + 1) * 128]), r_(gw_sb),
                    start=True, stop=True)

            def g_finish(xc):
                # 512-wide prelu chain + DRAM bounce for chunk xc
                ps_g8f = ps_g8s[xc].rearrange("p a b -> p (a b)")
                gv = gtmp.tile([128, 8, C], F32, tag="gv")
                gvf = gv.rearrange("p a b -> p (a b)")
                gsl = g_sb[:, xc * 8:(xc + 1) * 8, :].rearrange(
                    "p a b -> p (a b)")
                nc.vector.tensor_scalar_min(gvf, ps_g8f, 0.0)
                # the alpha mul is SBUF-only, so gpsimd can carry it
                nc.gpsimd.tensor_mul(gvf, gvf, ga8)
                nc.vector.scalar_tensor_tensor(
                    out=gsl, in0=ps_g8f, scalar=0.0, in1=gvf,
                    op0=Alu.max, op1=Alu.add)
                for p1 in range(2):
                    nc.sync.dma_start(
                        out=glint[p1, :, xc * 8:(xc + 1) * 8, :],
                        in_=g_sb[p1 * 64:(p1 + 1) * 64,
                                 xc * 8:(xc + 1) * 8, :])
                for hr in range(4):
                    nc.sync.dma_start(
                        out=g_poly[1 + 4 * xc:5 + 4 * xc, 1:17, hr, :],
                        in_=gl5[4 * xc:4 * xc + 4, hr, :, :])

            ps_x2 = [None]
            for xc in range(4):
                for t in range(xc * 8, (xc + 1) * 8):
                    # two transposes share one psum tile so the
                    # psum->SBUF drain is one 256-wide copy per pair
                    if t % 2 == 0:
                        ps_x2[0] = ps_x.tile([C, 256], F32, tag="x",
                                             name=f"ps_x{t}")
                    half = ps_x2[0][:, (t % 2) * 128:(t % 2) * 128 + 128]
                    nc.tensor.transpose(half, xP[:, t, :], ident)
                    if t % 2 == 1:
                        dst = r_(xTf[:C, (t - 1) * 128:(t + 1) * 128])
                        if t % 4 == 1:
                            nc.scalar.copy(out=dst, in_=ps_x2[0])
                        else:
                            nc.vector.tensor_copy(out=dst, in_=ps_x2[0])
                    if t >= 1:
                        g_mm(t - 1)
                    if t % 8 == 0 and t >= 8:
                        g_finish(xc - 1)
                    keepalive(1)
                # bilinear 4-tap partial for this xc's coarse rows
                hq = slice(4 * xc, 4 * xc + 4)
                nc.gpsimd.tensor_add(r_(phi_inT[:C, hq, :]),
                                     xv[:C, hq, 1, :, 1],
                                     xv[:C, hq, 1, :, 2])
                nc.gpsimd.tensor_add(r_(phi_inT[:C, hq, :]),
                                     phi_inT[:C, hq, :],
                                     xv[:C, hq, 2, :, 1])
                nc.gpsimd.tensor_add(r_(phi_inT[:C, hq, :]),
                                     phi_inT[:C, hq, :],
                                     xv[:C, hq, 2, :, 2])
                theta_chunk(2 * xc)
                theta_chunk(2 * xc + 1)
                if xc == 0:
                    packs(0)
                    packs(1)
                keepalive(6)
            g_mm(31)
            g_finish(3)
            keepalive(8)

            # phi: packed 1x1 conv + prelu into a 4x-replicated padded
            # plane (the 0.25 bilinear scale lives in phw4)
            ps_phi = ps_t.tile([128, N], F32, tag="t")
            nc.tensor.matmul(
                ps_phi, r_(phw4), r_(phi_inT.rearrange("c a b -> c (a b)")),
                start=True, stop=True)
            keepalive(4)
            nc.scalar.activation(
                out=phi4_pad[:, 1:17, 1:17],
                in_=ps_phi.rearrange("p (a b) -> p a b", b=WS),
                func=Act.Prelu, alpha=pha4)

            # per-patch L2 norm (group 0 holds a full phi copy);
            # separable 3x3 box sum: 2 row adds + 2 col adds
            sq = ttmp.tile([CI, 324], F32, tag="sq")
            nc.scalar.activation(r_(sq),
                                 phi4_pad[:CI].rearrange("p a b -> p (a b)"),
                                 Act.Square)
            ps_n2 = ps_t.tile([1, 324], F32, tag="t")
            nc.tensor.matmul(ps_n2, r_(ones32), r_(sq), start=True, stop=True)
            nc.scalar.copy(out=n2p, in_=ps_n2)
            keepalive(20)
            n2v = n2p.rearrange("p (a b) -> p a b", b=18)
            nrm3 = nrm.rearrange("p (a b) -> p a b", b=WS)
            nc.vector.tensor_add(n2row, n2v[:, 0:16, :], n2v[:, 1:17, :])
            nc.vector.tensor_add(n2row, n2row, n2v[:, 2:18, :])
            nc.vector.tensor_add(nrm3, n2row[:, :, 0:16], n2row[:, :, 1:17])
            nc.vector.tensor_add(nrm3, nrm3, n2row[:, :, 2:18])
            # 10/max(sqrt(n2),1e-6) == exp(-0.5*ln(max(n2,1e-12))+ln10);
            # keeps every scalar fn in one ACT table set (no sqrt)
            nc.vector.tensor_scalar_max(nrm, nrm, 1e-12)
            nc.scalar.activation(out=nrm, in_=nrm, func=Act.Ln)
            nc.scalar.activation(out=nrm, in_=nrm, func=Act.Exp,
                                 scale=-0.5, bias=ln10)
            # broadcast 10/||phi|| across partitions via a K=1 matmul
            # (stays in psum; the extract muls read it from there;
            # reuses the rotating "t" psum tag to stay within 8 banks)
            s10full = ps_t.tile([128, 512], F32, tag="t")
            s10ps = s10full[:, :N]
            nc.tensor.matmul(s10ps, ones_row, nrm,
                             start=True, stop=True)
            keepalive(16)
            s10v = s10ps.rearrange("p (a b) -> p a b", b=WS)

            # extract phi tap packs (partition-aligned shifted windows)
            # as muls, folding the softmax scale in on the way
            fp_flat = [fpA, fpB]
            for t, (kh, kw) in enumerate(TAPS):
                P, g = t // 4, t % 4
                sl = slice(32 * g, 32 * g + 32)
                dst = fpC if P == 2 else fp_flat[P][sl]
                nc.vector.tensor_mul(
                    dst.rearrange("p (a b) -> p a b", b=WS),
                    phi4_pad[sl, kh:kh + 16, kw:kw + 16],
                    s10v[sl])

        # ---- stage 2: fused scores/softmax/deconv pipeline ----
        with ExitStack() as st2:
            e2 = st2.enter_context
            kgp = e2(tc.tile_pool(name="kgp", bufs=1))
            schp = e2(tc.tile_pool(name="schp", bufs=2))
            rbp = e2(tc.tile_pool(name="rbp", bufs=1, space="PSUM"))
            ps_d = e2(tc.tile_pool(name="ps_d", bufs=3, space="PSUM"))

            # gather the 18 dynamic-filter tiles from g_poly, one DMA
            # each, issued from the scalar engine's DMA queue so they
            # don't sit behind stage 1's ~50 sync-queue descriptors
            # kg[q,qw,kb][(i,j), r, (rw c)] = g_poly[i+kb*8+q, j+qw, r, :]
            kg = {}
            for q in range(3):
                for qw in range(3):
                    for kb in range(2):
                        t_ = kgp.tile([128, 4, 256], BF16,
                                      tag=f"kg{q}{qw}{kb}",
                                      name=f"kg{q}{qw}{kb}")
                        gsrc = g_poly[kb * 8 + q: kb * 8 + q + 8,
                                      qw: qw + 16, :, :]
                        nc.sync.dma_start(out=t_, in_=gsrc)
                        kg[(q, qw, kb)] = t_

            def phase_a(ch):
                h0 = ch * 8
                for kb in range(2):
                    ps_s = ps_sc.tile([128, 512], F32, tag="sc",
                                      name=f"ps_s{ch}_{kb}")
                    nc.tensor.matmul(
                        ps_s, fpA[:, kb * 128:(kb + 1) * 128],
                        thpA[:, h0:h0 + 8, :],
                        start=True, stop=False)
                    nc.tensor.matmul(
                        ps_s, fpB[:, kb * 128:(kb + 1) * 128],
                        thpB[:, h0:h0 + 8, :],
                        start=False, stop=False)
                    nc.tensor.matmul(
                        ps_s, fpC[:, kb * 128:(kb + 1) * 128],
                        thpC[:, h0:h0 + 8, :],
                        start=False, stop=True)
                    nc.scalar.activation(
                        out=e_t[ch][:, kb],
                        in_=ps_s.rearrange("p (a b) -> p a b", b=64),
                        func=Act.Exp)

            def phase_b(ch):
                h0 = ch * 8
                ps_S = ps_Sp.tile([1, 512], F32, tag="S", name=f"ps_S{ch}")
                for kb in range(2):
                    nc.tensor.matmul(
                        ps_S, sixes128, e_t[ch][:, kb],
                        start=(kb == 0), stop=(kb == 1))
                sch = schp.tile([1, 512], F32, tag="sch", name=f"sch{ch}")
                nc.vector.reciprocal_approx_fast(out=sch, in_=ps_S)
                # f32r-rounded copy so the K=1 broadcast matmul streams
                # at 1 cyc/row (plain fp32 pays 4x = 853ns of PE issue)
                sch_r = schp.tile([1, 512], F32, tag="schr",
                                  name=f"schr{ch}")
                nc.vector.tensor_copy(out=r_(sch_r), in_=sch)
                rb_t = rbp.tile([128, 512], F32, tag="rb", name=f"rb{ch}")
                nc.tensor.matmul(rb_t, r_(ones_row), r_(sch_r),
                                 start=True, stop=True)
                rb3 = rb_t.rearrange("p (a b) -> p a b", b=64)
                for kb in range(2):
                    nc.vector.tensor_mul(
                        attn_q1n[:, kb, 1 + h0:9 + h0, :],
                        e_t[ch][:, kb], rb3)
                    nc.vector.tensor_mul(
                        attn_q[0][:, kb, 1 + h0:9 + h0, 0:63],
                        e_t[ch][:, kb, :, 1:64], rb3[:, :, 1:64])
                    nc.vector.tensor_mul(
                        attn_q[2][:, kb, 1 + h0:9 + h0, 1:64],
                        e_t[ch][:, kb, :, 0:63], rb3[:, :, 0:63])

            # y viewed as [hq, wq, r, (rw c)] for the output-major drain
            y_r2 = y_h.ap().rearrange(
                "(hq r) (wq rw) c -> hq wq r (rw c)", r=4, rw=4)
            dp = [attn_q[0], attn_q1n, attn_q[2]]

            def deconv_pc(pc, pxcs=(0, 1, 2, 3)):
                for pxc in pxcs:
                    hp = 8 * pc + 2 * pxc
                    for rh in range(2):
                        ps_o = ps_d.tile([128, 512], F32, tag="d",
                                         name=f"ps_o{pc}_{pxc}_{rh}")
                        first = True
                        for q in range(3):
                            for qw in range(3):
                                for kb in range(2):
                                    nc.tensor.matmul(
                                        ps_o,
                                        dp[qw][:, kb,
                                               hp + 2 - q:hp + 4 - q, :],
                                        kg[(q, qw, kb)][:,
                                                        2 * rh:2 * rh + 2, :],
                                        start=first,
                                        stop=(q == 2 and qw == 2 and kb == 1))
                                    first = False
                        st_ = staging.tile([128, 2, 256], BF16, tag="stg",
                                           name=f"st{pc}_{pxc}_{rh}")
                        nc.scalar.copy(
                            out=st_.rearrange("p a b -> p (a b)"), in_=ps_o)
                        for a in range(2):
                            hq = pc * 8 + pxc * 2 + a
                            nc.sync.dma_start(
                                out=y_r2[hq, :, 2 * rh:2 * rh + 2, :],
                                in_=st_[a * 64:(a + 1) * 64])

            # lag-2 pipeline with deferred packs at prefetch distance 2:
            # scores/Exp(ch) -> first half of deconv(ch-2) ->
            # packs(ch+2) -> B(ch-1) -> second half of deconv(ch-2).
            # h2(pc) needs B(pc+1), issued just before it and hidden by
            # the ~17us of h1+scores matmuls in front of it on the PE.
            for ch in range(8):
                phase_a(ch)
                if ch >= 2:
                    deconv_pc(ch - 2, (0, 1))
                if ch + 2 < 8:
                    packs(ch + 2)
                if ch >= 1:
                    phase_b(ch - 1)
                if ch >= 2:
                    deconv_pc(ch - 2, (2, 3))
                else:
                    # no deconv yet: keep the PE dense so the HAM clock
                    # gate stays open through the pipeline head
                    keepalive(20)
                if ch in (2, 3):
                    # the first deconvs are paced by kg arrival and the
                    # B chains; filler keeps the clock grant through the
                    # ramp until the stream is self-sustaining
                    keepalive(10)
            deconv_pc(6, (0, 1))
            phase_b(7)
            deconv_pc(6, (2, 3))
            deconv_pc(7)

    nc.finalize()
    return nc


def kernel(**inputs):
    from concourse.bass_utils import run_bass_kernel_spmd

    if "nc" not in _CACHE:
        _CACHE["nc"] = _build_nc()
    nc = _CACHE["nc"]

    arrs = {k: np.ascontiguousarray(np.asarray(v, dtype=np.float32))
            for k, v in inputs.items()}
    x = arrs.pop("x")
    in_maps = [dict(arrs, x=x[b]) for b in range(B)]
    res = run_bass_kernel_spmd(nc, in_maps, core_ids=list(range(B)))
    return np.stack([np.asarray(res.results[b]["y"]).astype(np.float32)
                     for b in range(B)])


# revision 53
# speedup vs baseline: 1.0031x; 1.0031x over previous
"""Cross-Scale Non-Local Attention kernel for 8x Trainium2 NeuronCores.

Data-parallel over batch: each of the 8 cores processes one sample
(B=8, H=W=64, C=64). Per-core Bass/Tile program (v2 schedule):

  1. x loaded in 4 chunks; each chunk PE-transposed to channel-major
     xT [65, 4096] whose last partition row is ones, so every 1x1 conv
     bias folds into its matmul (K=65).
  2. g = prelu(xT.T @ g_w) bounced per-x-chunk to DRAM as bf16 g_poly
     (polyphase layout, zero border ring); the 18 shifted dynamic-filter
     views kg[q,qw,kb] [n=128, 4, 256] are gathered back by strided DMA.
  3. theta 1x1 convs (4x-replicated weights) produce per-chunk t_act
     tiles; the 9 shift-baked pack copies into thetaPackA/B/C are
     DEFERRED into the deconv pipeline (prefetch distance 2) where the
     DVE/GpSimd/Scalar engines are otherwise idle, instead of forming a
     60us vector-bound wall before the deconv as in v1.
  4. phi is accumulated per-x-chunk during the transposes (bilinear
     4-tap partials), its 0.25 scale folded into phi_w; the patch-norm
     3x3 box sum is separable (4 adds), the reciprocal uses the fast
     approx DVE op. phi tap packs get the softmax scale folded in.
  5. Main pipeline, lag-2 (deconv(pc) tile-waits on every issued B;
     each B's off-PE chain is hidden by the h1+scores matmuls in front
     of its consumer): per iteration
     scores/Exp(ch) -> deconv(ch-2) first half -> packs(ch+2) ->
     B(ch-1) -> deconv(ch-2) second half. The softmax denominator
     broadcast is a K=1 matmul into psum (gpsimd partition_broadcast
     costs ~7us in program switches), the reciprocal is the fast
     approx DVE op, and keepalive matmuls keep the HAM clock gate open
     until the deconv stream sustains it.
  6. Deconv as polyphase conv-transpose: psum[pix 128, rc 512]
     accumulates 18 shifted bf16 matmuls; drain = one scalar copy
     (fp32->bf16) + two output DMAs per tile (y is written bf16 and
     widened on host, halving drain bytes and DMA descriptor size).
"""

import numpy as np

_CACHE = {}

# Problem constants (hardcoded per harness contract)
B = 8
H = W = 64
C = 64
CI = 32
HS = WS = 16
N = 256          # HS*WS low-res positions
PH = 66          # padded attn spatial extent (64 + 1 halo each side)

# tap order for the packed scores operands: packs A,B hold 4 taps each
# on partition groups 0-3; pack C holds the 9th tap on partitions 0-31
TAPS = [(0, 0), (0, 1), (0, 2), (1, 0), (1, 1), (1, 2), (2, 0), (2, 1),
        (2, 2)]


def _build_nc():
    import concourse.bass as bass
    import concourse.tile as tile
    from concourse import bacc, mybir
    from concourse.masks import make_identity
    from contextlib import ExitStack

    F32 = mybir.dt.float32
    F32R = mybir.dt.float32r
    BF16 = mybir.dt.bfloat16
    Alu = mybir.AluOpType
    Act = mybir.ActivationFunctionType

    def r_(ap):
        return ap.bitcast(F32R)

    nc = bacc.Bacc("TRN2", debug=False)

    x_h = nc.dram_tensor("x", [H, W, C], F32, kind="ExternalInput")
    thw_h = nc.dram_tensor("theta_w", [C, CI], F32, kind="ExternalInput")
    thb_h = nc.dram_tensor("theta_b", [CI], F32, kind="ExternalInput")
    tha_h = nc.dram_tensor("theta_alpha", [CI], F32, kind="ExternalInput")
    phw_h = nc.dram_tensor("phi_w", [C, CI], F32, kind="ExternalInput")
    phb_h = nc.dram_tensor("phi_b", [CI], F32, kind="ExternalInput")
    pha_h = nc.dram_tensor("phi_alpha", [CI], F32, kind="ExternalInput")
    gw_h = nc.dram_tensor("g_w", [C, C], F32, kind="ExternalInput")
    gb_h = nc.dram_tensor("g_b", [C], F32, kind="ExternalInput")
    ga_h = nc.dram_tensor("g_alpha", [C], F32, kind="ExternalInput")
    y_h = nc.dram_tensor("y", [4 * H, 4 * W, C], BF16, kind="ExternalOutput")

    with tile.TileContext(nc) as tc, ExitStack() as top:
        ec = top.enter_context

        consts = ec(tc.tile_pool(name="consts", bufs=1))
        xp_pool = ec(tc.tile_pool(name="xp_pool", bufs=1))
        persist = ec(tc.tile_pool(name="persist", bufs=1))
        phip = ec(tc.tile_pool(name="phip", bufs=1))
        dramp = ec(tc.tile_pool(name="dramp", bufs=1, space="DRAM"))
        staging = ec(tc.tile_pool(name="staging", bufs=3))
        ps_sc = ec(tc.tile_pool(name="ps_sc", bufs=2, space="PSUM"))
        ps_Sp = ec(tc.tile_pool(name="ps_Sp", bufs=1, space="PSUM"))
        ps_ka = ec(tc.tile_pool(name="ps_ka", bufs=1, space="PSUM"))

        # ---- x first on the DMA queues ----
        xP = xp_pool.tile([128, 32, C], F32)
        x_r = x_h.ap().rearrange("h w c -> (h w) c").rearrange(
            "(t p) c -> p t c", p=128)
        for xc in range(4):
            nc.sync.dma_start(
                out=xP[:, xc * 8:(xc + 1) * 8, :],
                in_=x_r[:, xc * 8:(xc + 1) * 8, :])

        # ---- constants / weights in SBUF ----
        ident = consts.tile([128, 128], F32)
        make_identity(nc, ident)
        # HAM warmup: bf16 matmuls keep the PE busy through the initial
        # x-load DMA so the clock gate is released when real work starts.
        wu = consts.tile([128, 512], BF16)
        nc.vector.memset(wu, 0.0)
        ps_wu = ps_ka.tile([128, 512], F32, tag="ka", name="ps_wu")
        for i in range(16):
            nc.tensor.matmul(ps_wu, wu[:, :128], wu, start=True, stop=True)

        ka_src = [wu]

        def keepalive(n):
            for _ in range(n):
                nc.tensor.matmul(ps_wu, ka_src[0][:, :128], ka_src[0],
                                 start=True, stop=True)

        # weights with the bias as a 65th contraction row; theta/phi
        # replicated 4x along the output dim so the PE emits the
        # (tap-group, ci) packed partitions directly
        # weight loads go out on the scalar engine's DMA queue so the
        # sync queue reaches the g bounce + kg gathers sooner
        thw4 = consts.tile([C + 1, 128], F32)
        phw4 = consts.tile([C + 1, 128], F32)
        gw_sb = consts.tile([C + 1, C], F32)
        thw_sb = consts.tile([C, CI], F32)
        nc.scalar.dma_start(out=r_(thw_sb), in_=r_(thw_h.ap()))
        phw_sb = consts.tile([C, CI], F32)
        nc.scalar.dma_start(out=r_(phw_sb), in_=r_(phw_h.ap()))
        nc.scalar.dma_start(out=r_(gw_sb[:C]), in_=r_(gw_h.ap()))
        nc.scalar.dma_start(out=r_(gw_sb[C:C + 1, :]),
                            in_=r_(gb_h.ap().unsqueeze(0)))
        tha4 = consts.tile([128, 1], F32)
        pha4 = consts.tile([128, 1], F32)
        for g in range(4):
            sl = slice(32 * g, 32 * g + 32)
            nc.scalar.dma_start(out=tha4[sl], in_=tha_h.ap().unsqueeze(1))
            nc.scalar.dma_start(out=pha4[sl], in_=pha_h.ap().unsqueeze(1))
            nc.scalar.dma_start(out=r_(thw4[C:C + 1, sl]),
                                in_=r_(thb_h.ap().unsqueeze(0)))
            nc.scalar.dma_start(out=r_(phw4[C:C + 1, sl]),
                                in_=r_(phb_h.ap().unsqueeze(0)))
        for g in range(4):
            nc.vector.tensor_copy(out=r_(thw4[:C, 32 * g:32 * g + 32]),
                                  in_=thw_sb)
            nc.vector.tensor_copy(out=r_(phw4[:C, 32 * g:32 * g + 32]),
                                  in_=phw_sb)
        # fold the bilinear 0.25 into phi_w (not the bias row)
        nc.vector.tensor_scalar_mul(r_(phw4[:C]), phw4[:C], 0.25)
        ga_row = consts.tile([1, C], F32)
        nc.scalar.dma_start(out=ga_row, in_=ga_h.ap().unsqueeze(0))
        ga_row8 = consts.tile([1, 8 * C], F32)
        for t in range(8):
            nc.vector.tensor_copy(out=ga_row8[:, t * C:(t + 1) * C],
                                  in_=ga_row)
        ga8 = consts.tile([128, 8 * C], F32)
        nc.gpsimd.partition_broadcast(ga8, ga_row8)
        sixes128 = consts.tile([128, 1], BF16)
        nc.vector.memset(sixes128, 6.0)
        ones32 = consts.tile([CI, 1], F32)
        nc.vector.memset(ones32, 1.0)
        # [1,128] ones: K=1 matmul broadcasts a row across partitions
        # (the gpsimd partition_broadcast costs ~7us in program switches)
        ones_row = consts.tile([1, 128], F32)
        nc.vector.memset(ones_row, 1.0)
        ln10 = consts.tile([1, 1], F32)
        nc.vector.memset(ln10, 2.302585092994046)
        ztb16 = consts.tile([128, 1024], BF16)
        nc.vector.memset(ztb16, 0.0)

        # polyphase g layouts in DRAM; the zero border ring only needs
        # ztb16, so its fills go out with the startup DMAs
        g_poly = dramp.tile([18, 18, 4, 256], BF16)
        g_lin = dramp.tile([H, W, C], BF16)
        gp_r0 = g_poly[0].rearrange("b r x -> (b r x)").rearrange(
            "(p f) -> p f", f=512)
        nc.scalar.dma_start(out=gp_r0, in_=ztb16[:36, :512])
        gp_r1 = g_poly[17].rearrange("b r x -> (b r x)").rearrange(
            "(p f) -> p f", f=512)
        nc.scalar.dma_start(out=gp_r1, in_=ztb16[:36, :512])
        gp_c0 = g_poly[1:17, 0].rearrange("a r x -> a (r x)")
        nc.scalar.dma_start(out=gp_c0, in_=ztb16[:16])
        gp_c1 = g_poly[1:17, 17].rearrange("a r x -> a (r x)")
        nc.scalar.dma_start(out=gp_c1, in_=ztb16[:16])

        # ---- persistent activation buffers ----
        attn_q = []
        for qw in range(3):
            t_ = persist.tile([128, 2, PH, 64], BF16, tag=f"attnq{qw}",
                              name=f"attnq{qw}")
            attn_q.append(t_)
        attn_q1n = persist.tile([128, 2, PH, 64], BF16, tag="attnq1n",
                                name="attnq1n")
        # raw E for all chunks (read-only after its Exp writes, so no
        # WAR chains)
        e_t = [persist.tile([128, 2, 8, 64], BF16, tag=f"e{ch}",
                            name=f"e{ch}") for ch in range(8)]
        # per-chunk theta activations, consumed by the deferred packs
        t_act = [persist.tile([128, 8, W], BF16, tag=f"ta{ch}",
                              name=f"ta{ch}") for ch in range(8)]
        thpA = persist.tile([128, 64, 64], BF16)
        thpB = persist.tile([128, 64, 64], BF16)
        thpC = persist.tile([CI, 64, 64], BF16)
        thp = [thpA, thpB, thpC]
        fpA = persist.tile([128, N], BF16)
        fpB = persist.tile([128, N], BF16)
        fpC = persist.tile([CI, N], BF16)

        phi4_pad = phip.tile([128, 18, 18], F32)
        nc.vector.memset(phi4_pad, 0.0)
        n2p = phip.tile([1, 324], F32)
        n2row = phip.tile([1, 16, 18], F32)
        nrm = phip.tile([1, N], F32)

        # only the never-written border strips of the packs / attn
        # planes need zeroing; issued up front (engines are idle during
        # the x load) as small zero-copies spread over three engines
        def zero_strips():
            strips = []
            for P, tile_, (g_lo, g_hi) in ((0, thpA, (0, 4)),
                                           (1, thpB, (4, 8)),
                                           (2, thpC, (8, 9))):
                for t in range(g_lo, g_hi):
                    kh, kw = TAPS[t]
                    g = t % 4
                    sl = slice(32 * g, 32 * g + 32)
                    if kh == 0:
                        strips.append((tile_[sl, 0, :], 64))
                    if kh == 2:
                        strips.append((tile_[sl, 63, :], 64))
                    if kw == 0:
                        strips.append((tile_[sl, :, 0], 64))
                    if kw == 2:
                        strips.append((tile_[sl, :, 63], 64))
            for tile2 in (attn_q[0], attn_q1n, attn_q[2]):
                for kb in range(2):
                    strips.append((tile2[:, kb, 0, :], 64))
                    strips.append((tile2[:, kb, PH - 1, :], 64))
            for kb in range(2):
                strips.append((attn_q[0][:, kb, :, 63], PH))
                strips.append((attn_q[2][:, kb, :, 0], PH))
            for i, (dst, width) in enumerate(strips):
                src = wu[:dst.shape[0], :width]
                e = i % 3
                if e == 0:
                    nc.vector.tensor_copy(out=dst, in_=src)
                elif e == 1:
                    nc.gpsimd.tensor_copy(out=dst, in_=src)
                else:
                    nc.scalar.copy(out=dst, in_=src)

        zero_strips()

        # bake each tap's spatial shift into its pack slice; scalar is
        # ~2x faster than DVE at these and gpsimd is slowest, so the
        # split leans on scalar with DVE relief
        PACK_ENG = ['s', 'v', 's', 's', 'v', 's', 's', 'v', 's']

        def packs(ch):
            h0 = ch * 8
            for t, (kh, kw) in enumerate(TAPS):
                P, g = t // 4, t % 4
                R0 = max(0, h0 + 1 - kh)
                R1 = min(64, h0 + 9 - kh)
                C0 = max(0, 1 - kw)
                C1 = min(64, 65 - kw)
                rs = R0 + kh - h0 - 1
                cs = C0 + kw - 1
                sl = slice(32 * g, 32 * g + 32)
                dst = thp[P][sl, R0:R1, C0:C1]
                src = t_act[ch][sl, rs:rs + R1 - R0, cs:cs + C1 - C0]
                eng = PACK_ENG[t]
                if eng == 'v':
                    nc.vector.tensor_copy(out=dst, in_=src)
                elif eng == 'g':
                    nc.gpsimd.tensor_copy(out=dst, in_=src)
                else:
                    nc.scalar.copy(out=dst, in_=src)

        with ExitStack() as st1:
            e1 = st1.enter_context
            xt_pool = e1(tc.tile_pool(name="xt_pool", bufs=1))
            gsb_pool = e1(tc.tile_pool(name="gsb_pool", bufs=1))
            ttmp = e1(tc.tile_pool(name="ttmp", bufs=2))
            gtmp = e1(tc.tile_pool(name="gtmp", bufs=2))
            ps_x = e1(tc.tile_pool(name="ps_x", bufs=1, space="PSUM"))
            ps_t = e1(tc.tile_pool(name="ps_t", bufs=1, space="PSUM"))
            ps_g = e1(tc.tile_pool(name="ps_g", bufs=2, space="PSUM"))

            # xT row 64 is all-ones: the bias row of the K=65 matmuls
            xT = xt_pool.tile([C + 1, H, W], F32)
            xTf = xT.rearrange("c h w -> c (h w)")
            nc.vector.memset(xT[C:C + 1], 1.0)
            phi_inT = xt_pool.tile([C + 1, HS, WS], F32)
            nc.vector.memset(phi_inT[C:C + 1], 1.0)
            g_sb = gsb_pool.tile([128, 32, C], BF16)

            def theta_chunk(ch):
                # the whole prelu is one scalar-engine parametric relu
                # (alpha is per-partition), freeing the DVE for g
                ps_tt = ps_t.tile([128, 512], F32, tag="t",
                                  name=f"ps_t{ch}")
                nc.tensor.matmul(
                    ps_tt, r_(thw4), r_(xTf[:, ch * 512:(ch + 1) * 512]),
                    start=True, stop=True)
                nc.scalar.activation(
                    out=t_act[ch].rearrange("p a b -> p (a b)"),
                    in_=ps_tt, func=Act.Prelu, alpha=tha4)

            # interleaved: transpose(t) -> copy -> g-mm(t-1) per pixel
            # chunk, so the g matmuls fill the PE gap left by the
            # psum->SBUF copy latency; keepalives hold the HAM clock up
            xv = xT.rearrange("c (hq hs) (wq ws) -> c hq hs wq ws",
                              hs=4, ws=4)
            gl5 = g_lin.rearrange(
                "(hq hr) (wq wr) c -> hq hr wq (wr c)", hr=4, wr=4)
            glint = g_lin.rearrange("(t a) w c -> a w t c", a=2)
            ps_g8s = {}

            def g_mm(t):
                xc = t // 8
                if xc not in ps_g8s:
                    ps_g8s[xc] = ps_g.tile([128, 8, C], F32, tag="g",
                                           name=f"ps_g8{xc}")
                nc.tensor.matmul(
                    ps_g8s[xc][:, t % 8, :],
                    r_(xTf[:, t * 128:(t + 1) * 128]), r_(gw_sb),
                    start=True, stop=True)

            def g_finish(xc):
                # 512-wide prelu chain + DRAM bounce for chunk xc
                ps_g8f = ps_g8s[xc].rearrange("p a b -> p (a b)")
                gv = gtmp.tile([128, 8, C], F32, tag="gv")
                gvf = gv.rearrange("p a b -> p (a b)")
                gsl = g_sb[:, xc * 8:(xc + 1) * 8, :].rearrange(
                    "p a b -> p (a b)")
                nc.vector.tensor_scalar_min(gvf, ps_g8f, 0.0)
                # the alpha mul is SBUF-only, so gpsimd can carry it
                nc.gpsimd.tensor_mul(gvf, gvf, ga8)
                nc.vector.scalar_tensor_tensor(
                    out=gsl, in0=ps_g8f, scalar=0.0, in1=gvf,
                    op0=Alu.max, op1=Alu.add)
                for p1 in range(2):
                    nc.sync.dma_start(
                        out=glint[p1, :, xc * 8:(xc + 1) * 8, :],
                        in_=g_sb[p1 * 64:(p1 + 1) * 64,
                                 xc * 8:(xc + 1) * 8, :])
                for hr in range(4):
                    nc.sync.dma_start(
                        out=g_poly[1 + 4 * xc:5 + 4 * xc, 1:17, hr, :],
                        in_=gl5[4 * xc:4 * xc + 4, hr, :, :])

            ps_x2 = [None]
            for xc in range(4):
                for t in range(xc * 8, (xc + 1) * 8):
                    # two transposes share one psum tile so the
                    # psum->SBUF drain is one 256-wide copy per pair
                    if t % 2 == 0:
                        ps_x2[0] = ps_x.tile([C, 256], F32, tag="x",
                                             name=f"ps_x{t}")
                    half = ps_x2[0][:, (t % 2) * 128:(t % 2) * 128 + 128]
                    nc.tensor.transpose(half, xP[:, t, :], ident)
                    if t % 2 == 1:
                        dst = r_(xTf[:C, (t - 1) * 128:(t + 1) * 128])
                        if t % 4 == 1:
                            nc.scalar.copy(out=dst, in_=ps_x2[0])
                        else:
                            nc.vector.tensor_copy(out=dst, in_=ps_x2[0])
                    if t >= 1:
                        g_mm(t - 1)
                    if t % 8 == 0 and t >= 8:
                        g_finish(xc - 1)
                    keepalive(1)
                # bilinear 4-tap partial for this xc's coarse rows
                hq = slice(4 * xc, 4 * xc + 4)
                nc.gpsimd.tensor_add(r_(phi_inT[:C, hq, :]),
                                     xv[:C, hq, 1, :, 1],
                                     xv[:C, hq, 1, :, 2])
                nc.gpsimd.tensor_add(r_(phi_inT[:C, hq, :]),
                                     phi_inT[:C, hq, :],
                                     xv[:C, hq, 2, :, 1])
                nc.gpsimd.tensor_add(r_(phi_inT[:C, hq, :]),
                                     phi_inT[:C, hq, :],
                                     xv[:C, hq, 2, :, 2])
                theta_chunk(2 * xc)
                theta_chunk(2 * xc + 1)
                if xc == 0:
                    packs(0)
                    packs(1)
                    # the HAM grant tracks switching activity, and
                    # all-zero matmuls look idle to it: once x is in
                    # SBUF, later keepalives stream real data instead
                    # (wu itself stays zero for the border strips)
                    wu2 = gsb_pool.tile([128, 512], BF16)
                    nc.vector.tensor_copy(
                        out=wu2,
                        in_=xP.rearrange("p a b -> p (a b)")[:, :512])
                    ka_src[0] = wu2
                keepalive(6)
            g_mm(31)
            g_finish(3)
            keepalive(8)

            # phi: packed 1x1 conv + prelu into a 4x-replicated padded
            # plane (the 0.25 bilinear scale lives in phw4)
            ps_phi = ps_t.tile([128, N], F32, tag="t")
            nc.tensor.matmul(
                ps_phi, r_(phw4), r_(phi_inT.rearrange("c a b -> c (a b)")),
                start=True, stop=True)
            keepalive(4)
            nc.scalar.activation(
                out=phi4_pad[:, 1:17, 1:17],
                in_=ps_phi.rearrange("p (a b) -> p a b", b=WS),
                func=Act.Prelu, alpha=pha4)

            # per-patch L2 norm (group 0 holds a full phi copy);
            # separable 3x3 box sum: 2 row adds + 2 col adds
            sq = ttmp.tile([CI, 324], F32, tag="sq")
            nc.scalar.activation(r_(sq),
                                 phi4_pad[:CI].rearrange("p a b -> p (a b)"),
                                 Act.Square)
            ps_n2 = ps_t.tile([1, 324], F32, tag="t")
            nc.tensor.matmul(ps_n2, r_(ones32), r_(sq), start=True, stop=True)
            nc.scalar.copy(out=n2p, in_=ps_n2)
            keepalive(20)
            n2v = n2p.rearrange("p (a b) -> p a b", b=18)
            nrm3 = nrm.rearrange("p (a b) -> p a b", b=WS)
            nc.vector.tensor_add(n2row, n2v[:, 0:16, :], n2v[:, 1:17, :])
            nc.vector.tensor_add(n2row, n2row, n2v[:, 2:18, :])
            nc.vector.tensor_add(nrm3, n2row[:, :, 0:16], n2row[:, :, 1:17])
            nc.vector.tensor_add(nrm3, nrm3, n2row[:, :, 2:18])
            # 10/max(sqrt(n2),1e-6) == exp(-0.5*ln(max(n2,1e-12))+ln10);
            # keeps every scalar fn in one ACT table set (no sqrt)
            nc.vector.tensor_scalar_max(nrm, nrm, 1e-12)
            nc.scalar.activation(out=nrm, in_=nrm, func=Act.Ln)
            nc.scalar.activation(out=nrm, in_=nrm, func=Act.Exp,
                                 scale=-0.5, bias=ln10)
            # broadcast 10/||phi|| across partitions via a K=1 matmul
            # (stays in psum; the extract muls read it from there;
            # reuses the rotating "t" psum tag to stay within 8 banks)
            s10full = ps_t.tile([128, 512], F32, tag="t")
            s10ps = s10full[:, :N]
            nc.tensor.matmul(s10ps, ones_row, nrm,
                             start=True, stop=True)
            keepalive(16)
            s10v = s10ps.rearrange("p (a b) -> p a b", b=WS)

            # extract phi tap packs (partition-aligned shifted windows)
            # as muls, folding the softmax scale in on the way
            fp_flat = [fpA, fpB]
            for t, (kh, kw) in enumerate(TAPS):
                P, g = t // 4, t % 4
                sl = slice(32 * g, 32 * g + 32)
                dst = fpC if P == 2 else fp_flat[P][sl]
                nc.vector.tensor_mul(
                    dst.rearrange("p (a b) -> p a b", b=WS),
                    phi4_pad[sl, kh:kh + 16, kw:kw + 16],
                    s10v[sl])

        # ---- stage 2: fused scores/softmax/deconv pipeline ----
        with ExitStack() as st2:
            e2 = st2.enter_context
            kgp = e2(tc.tile_pool(name="kgp", bufs=1))
            schp = e2(tc.tile_pool(name="schp", bufs=2))
            rbp = e2(tc.tile_pool(name="rbp", bufs=1, space="PSUM"))
            ps_d = e2(tc.tile_pool(name="ps_d", bufs=3, space="PSUM"))

            # gather the 18 dynamic-filter tiles from g_poly, one DMA
            # each, issued from the scalar engine's DMA queue so they
            # don't sit behind stage 1's ~50 sync-queue descriptors
            # kg[q,qw,kb][(i,j), r, (rw c)] = g_poly[i+kb*8+q, j+qw, r, :]
            kg = {}
            for q in range(3):
                for qw in range(3):
                    for kb in range(2):
                        t_ = kgp.tile([128, 4, 256], BF16,
                                      tag=f"kg{q}{qw}{kb}",
                                      name=f"kg{q}{qw}{kb}")
                        gsrc = g_poly[kb * 8 + q: kb * 8 + q + 8,
                                      qw: qw + 16, :, :]
                        nc.sync.dma_start(out=t_, in_=gsrc)
                        kg[(q, qw, kb)] = t_

            def phase_a(ch):
                h0 = ch * 8
                for kb in range(2):
                    ps_s = ps_sc.tile([128, 512], F32, tag="sc",
                                      name=f"ps_s{ch}_{kb}")
                    nc.tensor.matmul(
                        ps_s, fpA[:, kb * 128:(kb + 1) * 128],
                        thpA[:, h0:h0 + 8, :],
                        start=True, stop=False)
                    nc.tensor.matmul(
                        ps_s, fpB[:, kb * 128:(kb + 1) * 128],
                        thpB[:, h0:h0 + 8, :],
                        start=False, stop=False)
                    nc.tensor.matmul(
                        ps_s, fpC[:, kb * 128:(kb + 1) * 128],
                        thpC[:, h0:h0 + 8, :],
                        start=False, stop=True)
                    nc.scalar.activation(
                        out=e_t[ch][:, kb],
                        in_=ps_s.rearrange("p (a b) -> p a b", b=64),
                        func=Act.Exp)

            def phase_b(ch):
                h0 = ch * 8
                ps_S = ps_Sp.tile([1, 512], F32, tag="S", name=f"ps_S{ch}")
                for kb in range(2):
                    nc.tensor.matmul(
                        ps_S, sixes128, e_t[ch][:, kb],
                        start=(kb == 0), stop=(kb == 1))
                sch = schp.tile([1, 512], F32, tag="sch", name=f"sch{ch}")
                nc.vector.reciprocal_approx_fast(out=sch, in_=ps_S)
                # f32r-rounded copy so the K=1 broadcast matmul streams
                # at 1 cyc/row (plain fp32 pays 4x = 853ns of PE issue)
                sch_r = schp.tile([1, 512], F32, tag="schr",
                                  name=f"schr{ch}")
                nc.vector.tensor_copy(out=r_(sch_r), in_=sch)
                rb_t = rbp.tile([128, 512], F32, tag="rb", name=f"rb{ch}")
                nc.tensor.matmul(rb_t, r_(ones_row), r_(sch_r),
                                 start=True, stop=True)
                rb3 = rb_t.rearrange("p (a b) -> p a b", b=64)
                for kb in range(2):
                    nc.vector.tensor_mul(
                        attn_q1n[:, kb, 1 + h0:9 + h0, :],
                        e_t[ch][:, kb], rb3)
                    nc.vector.tensor_mul(
                        attn_q[0][:, kb, 1 + h0:9 + h0, 0:63],
                        e_t[ch][:, kb, :, 1:64], rb3[:, :, 1:64])
                    nc.vector.tensor_mul(
                        attn_q[2][:, kb, 1 + h0:9 + h0, 1:64],
                        e_t[ch][:, kb, :, 0:63], rb3[:, :, 0:63])

            # y viewed as [hq, wq, r, (rw c)] for the output-major drain
            y_r2 = y_h.ap().rearrange(
                "(hq r) (wq rw) c -> hq wq r (rw c)", r=4, rw=4)
            dp = [attn_q[0], attn_q1n, attn_q[2]]

            def deconv_pc(pc, pxcs=(0, 1, 2, 3)):
                for pxc in pxcs:
                    hp = 8 * pc + 2 * pxc
                    for rh in range(2):
                        ps_o = ps_d.tile([128, 512], F32, tag="d",
                                         name=f"ps_o{pc}_{pxc}_{rh}")
                        first = True
                        for q in range(3):
                            for qw in range(3):
                                for kb in range(2):
                                    nc.tensor.matmul(
                                        ps_o,
                                        dp[qw][:, kb,
                                               hp + 2 - q:hp + 4 - q, :],
                                        kg[(q, qw, kb)][:,
                                                        2 * rh:2 * rh + 2, :],
                                        start=first,
                                        stop=(q == 2 and qw == 2 and kb == 1))
                                    first = False
                        st_ = staging.tile([128, 2, 256], BF16, tag="stg",
                                           name=f"st{pc}_{pxc}_{rh}")
                        nc.scalar.copy(
                            out=st_.rearrange("p a b -> p (a b)"), in_=ps_o)
                        for a in range(2):
                            hq = pc * 8 + pxc * 2 + a
                            nc.sync.dma_start(
                                out=y_r2[hq, :, 2 * rh:2 * rh + 2, :],
                                in_=st_[a * 64:(a + 1) * 64])

            # lag-2 pipeline with deferred packs at prefetch distance 2:
            # scores/Exp(ch) -> first half of deconv(ch-2) ->
            # packs(ch+2) -> B(ch-1) -> second half of deconv(ch-2).
            # h2(pc) needs B(pc+1), issued just before it and hidden by
            # the ~17us of h1+scores matmuls in front of it on the PE.
            for ch in range(8):
                phase_a(ch)
                if ch >= 2:
                    deconv_pc(ch - 2, (0, 1))
                if ch + 2 < 8:
                    packs(ch + 2)
                if ch >= 1:
                    phase_b(ch - 1)
                if ch >= 2:
                    deconv_pc(ch - 2, (2, 3))
                else:
                    # no deconv yet: keep the PE dense so the HAM clock
                    # gate stays open through the pipeline head
                    keepalive(20)
            deconv_pc(6, (0, 1))
            phase_b(7)
            deconv_pc(6, (2, 3))
            deconv_pc(7)

    nc.finalize()
    return nc


def kernel(**inputs):
    from concourse.bass_utils import run_bass_kernel_spmd

    if "nc" not in _CACHE:
        _CACHE["nc"] = _build_nc()
    nc = _CACHE["nc"]

    arrs = {k: np.ascontiguousarray(np.asarray(v, dtype=np.float32))
            for k, v in inputs.items()}
    x = arrs.pop("x")
    in_maps = [dict(arrs, x=x[b]) for b in range(B)]
    res = run_bass_kernel_spmd(nc, in_maps, core_ids=list(range(B)))
    return np.stack([np.asarray(res.results[b]["y"]).astype(np.float32)
                     for b in range(B)])
